# revision 1
# baseline (speedup 1.0000x reference)
"""Trainium2 Bass kernel for nn_DNM_Conv (LayerNorm -> synapse contraction ->
dendritic weighting -> GELU -> residual multiply).

Algebraic reduction of the reference:
    y = LayerNorm(x)                                  (b, n, d)
    t[b,o,d] = sum_n W[o,n] * y[b,n,d] + c[o]
        where W[o,n] = sum_m dw[o,m]*sw[o,m,n],  c[o] = sum_{m,n} dw[o,m]*sb[o,m,n]
    out = x * (gelu_erf(t) + 1)                       (o == n == 196)

Distribution: data-parallel over batch, 8 batches per core on 8 cores.
W^T and c are tiny and replicated. Matmul runs in float32r (tf32-like,
1 cycle/row at N>=256) with accumulation in fp32 PSUM.
"""

import numpy as np

B, N, D, O, M = 64, 196, 768, 196, 2
N_CORES = 8
BPC = B // N_CORES          # batches per core
NA, NB = 128, 68            # n partition split
OA, OB = 128, 68            # o partition split
DC = 384                    # matmul moving free-dim chunk (2 chunks of 384)
LN_EPS = 1e-5

_NC_CACHE = {}


def _build_nc(nontrivial_ln):
    import concourse.bacc as bacc
    import concourse.tile as tile
    from concourse import mybir
    from contextlib import ExitStack

    F32 = mybir.dt.float32
    F32R = mybir.dt.float32r
    AF = mybir.ActivationFunctionType
    OP = mybir.AluOpType

    nc = bacc.Bacc()
    x_d = nc.declare_dram_parameter("x", [BPC, N, D], F32, isOutput=False)
    wt_d = nc.declare_dram_parameter("wt", [N, O], F32, isOutput=False)
    c_d = nc.declare_dram_parameter("c", [O, 1], F32, isOutput=False)
    if nontrivial_ln:
        lnw_d = nc.declare_dram_parameter("lnw", [1, D], F32, isOutput=False)
        lnbe_d = nc.declare_dram_parameter("lnbe", [O, D], F32, isOutput=False)
    out_d = nc.declare_dram_parameter("out", [BPC, N, D], F32, isOutput=True)

    with tile.TileContext(nc) as tc, ExitStack() as ctx:
        const = ctx.enter_context(tc.tile_pool(name="const", bufs=1))
        xpool = ctx.enter_context(tc.tile_pool(name="xpool", bufs=3))
        ypool = ctx.enter_context(tc.tile_pool(name="ypool", bufs=2))
        spool = ctx.enter_context(tc.tile_pool(name="spool", bufs=3))
        gpool = ctx.enter_context(tc.tile_pool(name="gpool", bufs=3))
        opool = ctx.enter_context(tc.tile_pool(name="opool", bufs=2))
        psum = ctx.enter_context(tc.tile_pool(name="psum", bufs=2, space="PSUM"))

        # ---- constants (loaded once) ----
        wt_a = const.tile([NA, O], F32R, tag="wt_a")
        wt_b = const.tile([NB, O], F32R, tag="wt_b")
        nc.gpsimd.dma_start(out=wt_a[:], in_=wt_d[0:NA, :])  # f32->f32r cast DMA
        nc.gpsimd.dma_start(out=wt_b[:], in_=wt_d[NA:N, :])
        c_a = const.tile([OA, 1], F32, tag="c_a")
        c_b = const.tile([OB, 1], F32, tag="c_b")
        nc.sync.dma_start(out=c_a[:], in_=c_d[0:OA, :])
        nc.sync.dma_start(out=c_b[:], in_=c_d[OA:O, :])
        eps_t = const.tile([128, 1], F32, tag="eps")
        nc.vector.memset(eps_t[:], LN_EPS)
        if nontrivial_ln:
            lnw_t = const.tile([128, D], F32, tag="lnw")
            import concourse.bass as bass
            lnw_bcast = bass.AP(tensor=lnw_d.ap().tensor, offset=0,
                                ap=[[0, 128], [1, D]])
            nc.sync.dma_start(out=lnw_t[:], in_=lnw_bcast)
            lnbe_a = const.tile([OA, D], F32, tag="lnbe_a")
            lnbe_b = const.tile([OB, D], F32, tag="lnbe_b")
            nc.sync.dma_start(out=lnbe_a[:], in_=lnbe_d[0:OA, :])
            nc.sync.dma_start(out=lnbe_b[:], in_=lnbe_d[OA:O, :])

        nsplit = ((0, NA), (NA, NB))
        osplit = ((0, OA, c_a), (OA, OB, c_b))

        for i in range(BPC):
            # ---- load x[i] ----
            xs = []
            for ci, (p0, pn) in enumerate(nsplit):
                xt = xpool.tile([pn, D], F32, tag=f"x{ci}")
                nc.sync.dma_start(out=xt[:], in_=x_d[i, p0:p0 + pn, :])
                xs.append(xt)

            # ---- LayerNorm -> y (float32r) ----
            ys = []
            for ci, xt in enumerate(xs):
                pn = xt.shape[0]
                stats = spool.tile([pn, 2, 6], F32, tag=f"stats{ci}")
                xg = xt[:].rearrange("p (s f) -> p s f", s=2)
                for s in range(2):
                    nc.vector.bn_stats(out=stats[:, s, :], in_=xg[:, s, :])
                mv = spool.tile([pn, 2], F32, tag=f"mv{ci}")
                nc.vector.bn_aggr(out=mv[:], in_=stats[:])
                std = spool.tile([pn, 1], F32, tag=f"std{ci}")
                nc.scalar.activation(out=std[:], in_=mv[:, 1:2], func=AF.Sqrt,
                                     bias=eps_t[0:pn, :], scale=1.0)
                rstd = spool.tile([pn, 1], F32, tag=f"rstd{ci}")
                nc.vector.reciprocal(out=rstd[:], in_=std[:])
                y = ypool.tile([pn, D], F32R, tag=f"y{ci}")
                nc.vector.tensor_scalar(out=y[:], in0=xt[:],
                                        scalar1=mv[:, 0:1], scalar2=rstd[:],
                                        op0=OP.subtract, op1=OP.mult)
                ys.append(y)

            # ---- matmul + gelu + residual multiply ----
            outs = []
            for oc, (o0, on, c_t) in enumerate(osplit):
                out_t = opool.tile([on, D], F32, tag=f"out{oc}")
                outs.append(out_t)
                for dc in range(2):
                    ds = slice(dc * DC, (dc + 1) * DC)
                    pm = psum.tile([on, DC], F32, tag=f"pm{oc}{dc}")
                    for k, (wt_t, y) in enumerate(zip((wt_a, wt_b), ys)):
                        nc.tensor.matmul(pm[:], wt_t[:, o0:o0 + on], y[:, ds],
                                         start=(k == 0), stop=(k == 1))
                    if nontrivial_ln:
                        lnbe_t = lnbe_a if oc == 0 else lnbe_b
                        nc.vector.tensor_mul(out=pm[:], in0=pm[:],
                                             in1=lnw_t[0:on, ds])
                        nc.vector.tensor_add(out=pm[:], in0=pm[:],
                                             in1=lnbe_t[:, ds])
                    g = gpool.tile([on, DC], F32, tag=f"g{oc}{dc}")
                    nc.scalar.activation(out=g[:], in_=pm[:], func=AF.Gelu,
                                         bias=c_t[:], scale=1.0)
                    # out = (g + 1) * x
                    nc.vector.scalar_tensor_tensor(
                        out=out_t[:, ds], in0=g[:], scalar=1.0,
                        in1=xs[oc][:, ds], op0=OP.add, op1=OP.mult)

            for (p0, pn), out_t in zip(nsplit, outs):
                nc.sync.dma_start(out=out_d[i, p0:p0 + pn, :], in_=out_t[:])

    nc.compile()
    return nc


def kernel(x, ln_w, ln_b, sw, sb, dw, _trace=False):
    from concourse.bass_utils import run_bass_kernel_spmd

    x = np.ascontiguousarray(np.asarray(x, dtype=np.float32))
    ln_w = np.asarray(ln_w, dtype=np.float32)
    ln_b = np.asarray(ln_b, dtype=np.float32)
    sw = np.asarray(sw, dtype=np.float32)
    sb = np.asarray(sb, dtype=np.float32)
    dw = np.asarray(dw, dtype=np.float32)

    # Fold dendritic weights into the synapse contraction (host, ~0.1 ms).
    W = np.einsum("om,omn->on", dw, sw)            # (o, n)
    WT = np.ascontiguousarray(W.T)                 # (n, o)
    c = np.einsum("om,om->o", dw, sb.sum(-1)).astype(np.float32)[:, None]

    nontrivial_ln = not (np.all(ln_w == 1.0) and np.all(ln_b == 0.0))
    key = bool(nontrivial_ln)
    if key not in _NC_CACHE:
        _NC_CACHE[key] = _build_nc(nontrivial_ln)
    nc = _NC_CACHE[key]

    in_maps = []
    for i in range(N_CORES):
        m = {"x": x[i * BPC:(i + 1) * BPC], "wt": WT, "c": c}
        if nontrivial_ln:
            m["lnw"] = ln_w[None, :]
            m["lnbe"] = (W.sum(-1)[:, None] * ln_b[None, :]).astype(np.float32)
        in_maps.append(m)

    res = run_bass_kernel_spmd(nc, in_maps, core_ids=list(range(N_CORES)),
                               trace=_trace)
    out = np.concatenate([res.results[i]["out"] for i in range(N_CORES)], axis=0)
    if _trace:
        return out, res
    return out


# revision 2
# speedup vs baseline: 1.0336x; 1.0336x over previous
"""Trainium2 Bass kernel for nn_DNM_Conv (LayerNorm -> synapse contraction ->
dendritic weighting -> GELU -> residual multiply).

Algebraic reduction of the reference:
    y = LayerNorm(x)                                  (b, n, d)
    t[b,o,d] = sum_n W[o,n] * y[b,n,d] + c[o]
        where W[o,n] = sum_m dw[o,m]*sw[o,m,n],  c[o] = sum_{m,n} dw[o,m]*sb[o,m,n]
    out = x * (gelu_erf(t) + 1)                       (o == n == 196)

Distribution: data-parallel over batch, 8 batches per core on 8 cores.
W^T and c are tiny and replicated. Matmul in fp16 (1 cycle/row on the PE,
fp32 PSUM accumulation); LN stats in fp32.

Engine balance per batch: DVE: bn_stats/aggr/recip + normalize(b-tile) +
2 residual-multiply tiles; ACT: normalize(a-tile) + 4 gelu; Pool: 2
residual-multiply tiles (as mult+add); PE: 8 matmuls. All Sqrt activations
are grouped before all Gelus to avoid ACT table-set thrash.
"""

import numpy as np

B, N, D, O, M = 64, 196, 768, 196, 2
N_CORES = 8
BPC = B // N_CORES          # batches per core
NA, NB = 128, 68            # n partition split
OA, OB = 128, 68            # o partition split
DC = 384                    # matmul moving free-dim chunk
LN_EPS = 1e-5

_NC_CACHE = {}


def _build_nc(nontrivial_ln):
    import concourse.bacc as bacc
    import concourse.tile as tile
    import concourse.bass as bass
    from concourse import mybir
    from contextlib import ExitStack

    F32 = mybir.dt.float32
    F16 = mybir.dt.float16
    AF = mybir.ActivationFunctionType
    OP = mybir.AluOpType

    nc = bacc.Bacc()
    x_d = nc.declare_dram_parameter("x", [BPC, N, D], F32, isOutput=False)
    wt_d = nc.declare_dram_parameter("wt", [N, O], F16, isOutput=False)
    c_d = nc.declare_dram_parameter("c", [O, 1], F32, isOutput=False)
    if nontrivial_ln:
        lnw_d = nc.declare_dram_parameter("lnw", [1, D], F32, isOutput=False)
        lnbe_d = nc.declare_dram_parameter("lnbe", [O, D], F32, isOutput=False)
    out_d = nc.declare_dram_parameter("out", [BPC, N, D], F32, isOutput=True)

    with tile.TileContext(nc) as tc, ExitStack() as ctx:
        const = ctx.enter_context(tc.tile_pool(name="const", bufs=1))
        # x tiles stay resident for the whole kernel (stats phase + main phase)
        xpool = ctx.enter_context(tc.tile_pool(name="xpool", bufs=BPC))
        stpool = ctx.enter_context(tc.tile_pool(name="stpool", bufs=BPC))
        ypool = ctx.enter_context(tc.tile_pool(name="ypool", bufs=3))
        gpool = ctx.enter_context(tc.tile_pool(name="gpool", bufs=3))
        opool = ctx.enter_context(tc.tile_pool(name="opool", bufs=3))
        psum = ctx.enter_context(tc.tile_pool(name="psum", bufs=2, space="PSUM"))

        # ---- constants ----
        wt_a = const.tile([NA, O], F16, tag="wt_a")
        wt_b = const.tile([NB, O], F16, tag="wt_b")
        nc.sync.dma_start(out=wt_a[:], in_=wt_d[0:NA, :])
        nc.sync.dma_start(out=wt_b[:], in_=wt_d[NA:N, :])
        c_a = const.tile([OA, 1], F32, tag="c_a")
        c_b = const.tile([OB, 1], F32, tag="c_b")
        nc.sync.dma_start(out=c_a[:], in_=c_d[0:OA, :])
        nc.sync.dma_start(out=c_b[:], in_=c_d[OA:O, :])
        eps_t = const.tile([128, 1], F32, tag="eps")
        nc.vector.memset(eps_t[:], LN_EPS)
        if nontrivial_ln:
            lnw_t = const.tile([128, D], F32, tag="lnw")
            lnw_bcast = bass.AP(tensor=lnw_d.ap().tensor, offset=0,
                                ap=[[0, 128], [1, D]])
            nc.sync.dma_start(out=lnw_t[:], in_=lnw_bcast)
            lnbe_a = const.tile([OA, D], F32, tag="lnbe_a")
            lnbe_b = const.tile([OB, D], F32, tag="lnbe_b")
            nc.sync.dma_start(out=lnbe_a[:], in_=lnbe_d[0:OA, :])
            nc.sync.dma_start(out=lnbe_b[:], in_=lnbe_d[OA:O, :])

        nsplit = ((0, NA), (NA, NB))

        # ---- phase A: load x, bn stats ----
        xts = []   # [i][ci]
        mvs = []   # [i][ci] -> (pn, 2) mean/var
        for i in range(BPC):
            xi, mvi = [], []
            for ci, (p0, pn) in enumerate(nsplit):
                xt = xpool.tile([pn, D], F32, tag=f"x{ci}")
                nc.sync.dma_start(out=xt[:], in_=x_d[i, p0:p0 + pn, :])
                stats = stpool.tile([pn, 2, 6], F32, tag=f"stats{ci}")
                xg = xt[:].rearrange("p (s f) -> p s f", s=2)
                for s in range(2):
                    nc.vector.bn_stats(out=stats[:, s, :], in_=xg[:, s, :])
                mv = stpool.tile([pn, 2], F32, tag=f"mv{ci}")
                nc.vector.bn_aggr(out=mv[:], in_=stats[:])
                xi.append(xt)
                mvi.append(mv)
            xts.append(xi)
            mvs.append(mvi)

        # ---- all Sqrt activations grouped (one table set period) ----
        stds = []
        for i in range(BPC):
            si = []
            for ci, (p0, pn) in enumerate(nsplit):
                std = stpool.tile([pn, 1], F32, tag=f"std{ci}")
                nc.scalar.activation(out=std[:], in_=mvs[i][ci][:, 1:2],
                                     func=AF.Sqrt, bias=eps_t[0:pn, :], scale=1.0)
                si.append(std)
            stds.append(si)

        rstds, nmmus = [], []
        for i in range(BPC):
            ri, ni = [], []
            for ci, (p0, pn) in enumerate(nsplit):
                rstd = stpool.tile([pn, 1], F32, tag=f"rstd{ci}")
                nc.vector.reciprocal(out=rstd[:], in_=stds[i][ci][:])
                ri.append(rstd)
                if ci == 0:
                    # nmmu = -mu * rstd  (bias for the ACT-side normalize)
                    nmmu = stpool.tile([pn, 1], F32, tag=f"nmmu{ci}")
                    nc.vector.scalar_tensor_tensor(
                        out=nmmu[:], in0=mvs[i][ci][:, 0:1], scalar=-1.0,
                        in1=rstd[:], op0=OP.mult, op1=OP.mult)
                    ni.append(nmmu)
                else:
                    ni.append(None)
            rstds.append(ri)
            nmmus.append(ni)

        # ---- phase B: normalize, matmul, gelu, residual multiply ----
        osplit = ((0, OA, c_a), (OA, OB, c_b))
        for i in range(BPC):
            xs = xts[i]
            ys = []
            # a-tile normalize on ACT: y = Identity(x * rstd + (-mu*rstd))
            ya = ypool.tile([NA, D], F16, tag="ya")
            nc.scalar.activation(out=ya[:], in_=xs[0][:], func=AF.Identity,
                                 bias=nmmus[i][0][:], scale=rstds[i][0][:])
            ys.append(ya)
            # b-tile normalize on DVE: y = (x - mu) * rstd
            yb = ypool.tile([NB, D], F16, tag="yb")
            nc.vector.tensor_scalar(out=yb[:], in0=xs[1][:],
                                    scalar1=mvs[i][1][:, 0:1],
                                    scalar2=rstds[i][1][:],
                                    op0=OP.subtract, op1=OP.mult)
            ys.append(yb)

            outs = []
            for oc, (o0, on, c_t) in enumerate(osplit):
                out_t = opool.tile([on, D], F32, tag=f"out{oc}")
                outs.append(out_t)
                for dc in range(2):
                    ds = slice(dc * DC, (dc + 1) * DC)
                    pm = psum.tile([on, DC], F32, tag=f"pm{oc}{dc}")
                    for k, (wt_t, y) in enumerate(zip((wt_a, wt_b), ys)):
                        nc.tensor.matmul(pm[:], wt_t[:, o0:o0 + on], y[:, ds],
                                         start=(k == 0), stop=(k == 1))
                    if nontrivial_ln:
                        lnbe_t = lnbe_a if oc == 0 else lnbe_b
                        nc.vector.tensor_mul(out=pm[:], in0=pm[:],
                                             in1=lnw_t[0:on, ds])
                        nc.vector.tensor_add(out=pm[:], in0=pm[:],
                                             in1=lnbe_t[:, ds])
                    g = gpool.tile([on, DC], F32, tag=f"g{oc}{dc}")
                    nc.scalar.activation(out=g[:], in_=pm[:], func=AF.Gelu,
                                         bias=c_t[:], scale=1.0)
                    # out = (g + 1) * x ; split across DVE (dc 0) and Pool (dc 1)
                    if dc == 0:
                        nc.vector.scalar_tensor_tensor(
                            out=out_t[:, ds], in0=g[:], scalar=1.0,
                            in1=xs[oc][:, ds], op0=OP.add, op1=OP.mult)
                    else:
                        nc.gpsimd.tensor_mul(out=out_t[:, ds], in0=g[:],
                                             in1=xs[oc][:, ds])
                        nc.gpsimd.tensor_add(out=out_t[:, ds],
                                             in0=out_t[:, ds],
                                             in1=xs[oc][:, ds])

            for (p0, pn), out_t in zip(nsplit, outs):
                nc.sync.dma_start(out=out_d[i, p0:p0 + pn, :], in_=out_t[:])

    nc.compile()
    return nc


def kernel(x, ln_w, ln_b, sw, sb, dw, _trace=False):
    from concourse.bass_utils import run_bass_kernel_spmd

    x = np.ascontiguousarray(np.asarray(x, dtype=np.float32))
    ln_w = np.asarray(ln_w, dtype=np.float32)
    ln_b = np.asarray(ln_b, dtype=np.float32)
    sw = np.asarray(sw, dtype=np.float32)
    sb = np.asarray(sb, dtype=np.float32)
    dw = np.asarray(dw, dtype=np.float32)

    # Fold dendritic weights into the synapse contraction (host, ~0.1 ms).
    W = np.einsum("om,omn->on", dw, sw)            # (o, n)
    WT = np.ascontiguousarray(W.T.astype(np.float16))
    c = np.einsum("om,om->o", dw, sb.sum(-1)).astype(np.float32)[:, None]

    nontrivial_ln = not (np.all(ln_w == 1.0) and np.all(ln_b == 0.0))
    key = bool(nontrivial_ln)
    if key not in _NC_CACHE:
        _NC_CACHE[key] = _build_nc(nontrivial_ln)
    nc = _NC_CACHE[key]

    in_maps = []
    for i in range(N_CORES):
        m = {"x": x[i * BPC:(i + 1) * BPC], "wt": WT, "c": c}
        if nontrivial_ln:
            m["lnw"] = ln_w[None, :]
            m["lnbe"] = (W.sum(-1)[:, None] * ln_b[None, :]).astype(np.float32)
        in_maps.append(m)

    res = run_bass_kernel_spmd(nc, in_maps, core_ids=list(range(N_CORES)),
                               trace=_trace)
    out = np.concatenate([res.results[i]["out"] for i in range(N_CORES)], axis=0)
    if _trace:
        return out, res
    return out


# revision 6
# speedup vs baseline: 1.1355x; 1.0986x over previous
"""Trainium2 Bass kernel for nn_DNM_Conv (LayerNorm -> synapse contraction ->
dendritic weighting -> GELU -> residual multiply).

Algebraic reduction of the reference:
    y = LayerNorm(x)                                  (b, n, d)
    t[b,o,d] = sum_n W[o,n] * y[b,n,d] + c[o]
        where W[o,n] = sum_m dw[o,m]*sw[o,m,n],  c[o] = sum_{m,n} dw[o,m]*sb[o,m,n]
    out = x * (gelu_erf(t) + 1)                       (o == n == 196)

Instead of normalizing x, the per-row LN scale is folded into the weights:
    Wr[o,n] = W[o,n] * rstd[n]        (per batch, tiny 196x196 scale)
    t[o,d]  = (Wr @ x)[o,d] - (Wr @ mu)[o] + c[o]
The mu-correction rides as an extra rhs column (x tile has 769 columns, the
last one holding mu), so one widened matmul produces both t and Wr@mu.

Distribution: data-parallel over batch, 8 batches per core on 8 cores.
Datapath is fp16 end to end (x cast on host, fp32 PSUM accumulation,
fp16 output cast back on host); LN statistics are computed in fp32.
"""

import numpy as np

B, N, D, O, M = 64, 196, 768, 196, 2
N_CORES = 8
BPC = B // N_CORES          # batches per core
NPAIR = BPC // 2            # batch pairs (DMA granularity)
NA, NB = 128, 68            # n partition split
OA, OB = 128, 68            # o partition split
DC = 384                    # matmul moving free-dim chunk
LN_EPS = 1e-5

_NC_CACHE = {}


def _build_nc(nontrivial_ln):
    import concourse.bacc as bacc
    import concourse.tile as tile
    import concourse.bass as bass
    from concourse.tile import add_dep_helper
    from concourse import mybir
    from contextlib import ExitStack

    F32 = mybir.dt.float32
    F16 = mybir.dt.float16
    AF = mybir.ActivationFunctionType
    OP = mybir.AluOpType

    nc = bacc.Bacc()
    x_d = nc.declare_dram_parameter("x", [BPC, N, D], F16, isOutput=False)
    wt_d = nc.declare_dram_parameter("wt", [N, O], F32, isOutput=False)
    c_d = nc.declare_dram_parameter("c", [O, 1], F32, isOutput=False)
    if nontrivial_ln:
        lnw_d = nc.declare_dram_parameter("lnw", [1, D], F32, isOutput=False)
        lnbe_d = nc.declare_dram_parameter("lnbe", [O, D], F32, isOutput=False)
    out_d = nc.declare_dram_parameter("out", [BPC, N, D], F16, isOutput=True)

    x_pair = x_d.ap().rearrange("(q j) n d -> q n j d", j=2)    # (4, 196, 2, 768)
    out_pair = out_d.ap().rearrange("(q j) n d -> q n j d", j=2)

    with tile.TileContext(nc) as tc, ExitStack() as ctx:
        const = ctx.enter_context(tc.tile_pool(name="const", bufs=1))
        xpool = ctx.enter_context(tc.tile_pool(name="xpool", bufs=NPAIR))
        stpool = ctx.enter_context(tc.tile_pool(name="stpool", bufs=BPC))
        wrpool = ctx.enter_context(tc.tile_pool(name="wrpool", bufs=3))
        gpool = ctx.enter_context(tc.tile_pool(name="gpool", bufs=3))
        opool = ctx.enter_context(tc.tile_pool(name="opool", bufs=2))
        psum = ctx.enter_context(tc.tile_pool(name="psum", bufs=2, space="PSUM"))

        # ---- constants ----
        wt_a = const.tile([NA, O], F32, tag="wt_a")
        wt_b = const.tile([NB, O], F32, tag="wt_b")
        nc.sync.dma_start(out=wt_a[:], in_=wt_d[0:NA, :])
        nc.sync.dma_start(out=wt_b[:], in_=wt_d[NA:N, :])
        c_a = const.tile([OA, 1], F32, tag="c_a")
        c_b = const.tile([OB, 1], F32, tag="c_b")
        nc.sync.dma_start(out=c_a[:], in_=c_d[0:OA, :])
        nc.sync.dma_start(out=c_b[:], in_=c_d[OA:O, :])
        eps_t = const.tile([128, 1], F32, tag="eps")
        nc.vector.memset(eps_t[:], LN_EPS)
        if nontrivial_ln:
            lnw_t = const.tile([128, D], F32, tag="lnw")
            lnw_bcast = bass.AP(tensor=lnw_d.ap().tensor, offset=0,
                                ap=[[0, 128], [1, D]])
            nc.sync.dma_start(out=lnw_t[:], in_=lnw_bcast)
            lnbe_a = const.tile([OA, D], F32, tag="lnbe_a")
            lnbe_b = const.tile([OB, D], F32, tag="lnbe_b")
            nc.sync.dma_start(out=lnbe_a[:], in_=lnbe_d[0:OA, :])
            nc.sync.dma_start(out=lnbe_b[:], in_=lnbe_d[OA:O, :])

        nsplit = ((0, NA), (NA, NB))

        # ---- phase A: load x (paired, fp16, 769th column reserved for mu*rstd),
        #      bn stats ----
        xtiles = []  # [pair][ci] -> (pn, 2, 769) fp16
        mvs = []     # [batch][ci] -> (pn, 2) fp32 mean/var
        for q in range(NPAIR):
            xq = []
            for ci, (p0, pn) in enumerate(nsplit):
                xt = xpool.tile([pn, 2, D + 1], F16, tag=f"x{ci}")
                nc.gpsimd.dma_start(out=xt[:, :, 0:D],
                                    in_=x_pair[q, p0:p0 + pn, :, :])
                xq.append(xt)
            xtiles.append(xq)
        for i in range(BPC):
            q, j = divmod(i, 2)
            mvi = []
            for ci, (p0, pn) in enumerate(nsplit):
                xt = xtiles[q][ci]
                stats = stpool.tile([pn, 2, 6], F32, tag=f"stats{ci}")
                xg = xt[:, j, 0:D].rearrange("p (s f) -> p s f", s=2)
                for s in range(2):
                    nc.vector.bn_stats(out=stats[:, s, :], in_=xg[:, s, :])
                mv = stpool.tile([pn, 2], F32, tag=f"mv{ci}")
                nc.vector.bn_aggr(out=mv[:], in_=stats[:])
                mvi.append(mv)
            mvs.append(mvi)

        # ---- rstd for all batches (single ACT table-set period) ----
        rstds = []
        rstd_insts = []
        for i in range(BPC):
            ri = []
            for ci, (p0, pn) in enumerate(nsplit):
                rstd = stpool.tile([pn, 1], F32, tag=f"rstd{ci}")
                ins = nc.scalar.activation(out=rstd[:], in_=mvs[i][ci][:, 1:2],
                                           func=AF.Abs_reciprocal_sqrt,
                                           bias=eps_t[0:pn, :], scale=1.0)
                rstd_insts.append(ins)
                ri.append(rstd)
            rstds.append(ri)

        # mu*rstd into the 769th x column (fp16), and Wr = wt * rstd (fp16)
        wrs = []
        for i in range(BPC):
            q, j = divmod(i, 2)
            wri = []
            for ci, (p0, pn) in enumerate(nsplit):
                nc.vector.tensor_mul(out=xtiles[q][ci][:, j, D:D + 1],
                                     in0=mvs[i][ci][:, 0:1],
                                     in1=rstds[i][ci][:])
                wr = wrpool.tile([pn, O], F16, tag=f"wr{ci}")
                wt_t = wt_a if ci == 0 else wt_b
                nc.vector.tensor_scalar_mul(out=wr[:], in0=wt_t[:],
                                            scalar1=rstds[i][ci][:])
                wri.append(wr)
            wrs.append(wri)

        # ---- phase B: matmul + gelu + residual multiply ----
        osplit = ((0, OA, c_a), (OA, OB, c_b))
        first_gelu_logged = False
        outs = None
        for i in range(BPC):
            q, j = divmod(i, 2)
            xs = xtiles[q]
            if j == 0:
                out_a = opool.tile([NA, 2, D], F16, tag="out0")
                out_b = opool.tile([NB, 2, D], F16, tag="out1")
                outs = (out_a, out_b)

            for oc, (o0, on, c_t) in enumerate(osplit):
                # dc=1 widened matmul first: columns 384..768 plus the mu column
                pm1 = psum.tile([on, DC + 1], F32, tag=f"pm{oc}1")
                for k, wr in enumerate(wrs[i]):
                    nc.tensor.matmul(pm1[:], wr[:, o0:o0 + on],
                                     xs[k][:, j, DC:D + 1],
                                     start=(k == 0), stop=(k == 1))
                gbias = stpool.tile([on, 1], F32, tag=f"gb{oc}")
                nc.vector.tensor_tensor(out=gbias[:], in0=c_t[:],
                                        in1=pm1[:, DC:DC + 1],
                                        op=OP.subtract)
                pm0 = psum.tile([on, DC], F32, tag=f"pm{oc}0")
                for k, wr in enumerate(wrs[i]):
                    nc.tensor.matmul(pm0[:], wr[:, o0:o0 + on],
                                     xs[k][:, j, 0:DC],
                                     start=(k == 0), stop=(k == 1))

                for dc, pm in ((1, pm1), (0, pm0)):
                    ds = slice(dc * DC, (dc + 1) * DC)
                    if nontrivial_ln:
                        lnbe_t = lnbe_a if oc == 0 else lnbe_b
                        # t = lnw * (pm - Wr@mu) + lnbe + c
                        nc.vector.tensor_scalar_sub(out=pm[:, 0:DC],
                                                    in0=pm[:, 0:DC],
                                                    scalar1=pm1[:, DC:DC + 1])
                        nc.vector.tensor_mul(out=pm[:, 0:DC], in0=pm[:, 0:DC],
                                             in1=lnw_t[0:on, ds])
                        nc.vector.tensor_add(out=pm[:, 0:DC], in0=pm[:, 0:DC],
                                             in1=lnbe_t[:, ds])
                        g = gpool.tile([on, DC], F16, tag=f"g{oc}{dc}")
                        ins = nc.scalar.activation(out=g[:], in_=pm[:, 0:DC],
                                                   func=AF.Gelu, bias=c_t[:],
                                                   scale=1.0)
                    else:
                        g = gpool.tile([on, DC], F16, tag=f"g{oc}{dc}")
                        ins = nc.scalar.activation(out=g[:], in_=pm[:, 0:DC],
                                                   func=AF.Gelu, bias=gbias[:],
                                                   scale=1.0)
                    if not first_gelu_logged:
                        first_gelu_logged = True
                        add_dep_helper(ins.ins, rstd_insts[-1].ins, sync=True,
                                       reason="group sqrt-set before gelu-set")
                    # out = (g + 1) * x
                    out_t = outs[oc]
                    if oc == 1:
                        nc.gpsimd.tensor_mul(out=out_t[:, j, ds], in0=g[:],
                                             in1=xs[oc][:, j, ds])
                        nc.gpsimd.tensor_add(out=out_t[:, j, ds],
                                             in0=out_t[:, j, ds],
                                             in1=xs[oc][:, j, ds])
                    else:
                        nc.vector.scalar_tensor_tensor(
                            out=out_t[:, j, ds], in0=g[:], scalar=1.0,
                            in1=xs[oc][:, j, ds], op0=OP.add, op1=OP.mult)

            if j == 1:
                for ci, (p0, pn) in enumerate(nsplit):
                    nc.sync.dma_start(out=out_pair[q, p0:p0 + pn, :, :],
                                      in_=outs[ci][:])

    nc.compile()
    return nc


def kernel(x, ln_w, ln_b, sw, sb, dw, _trace=False):
    from concourse.bass_utils import run_bass_kernel_spmd

    x = np.asarray(x, dtype=np.float32)
    ln_w = np.asarray(ln_w, dtype=np.float32)
    ln_b = np.asarray(ln_b, dtype=np.float32)
    sw = np.asarray(sw, dtype=np.float32)
    sb = np.asarray(sb, dtype=np.float32)
    dw = np.asarray(dw, dtype=np.float32)

    x16 = np.ascontiguousarray(x.astype(np.float16))

    # Fold dendritic weights into the synapse contraction (host, ~0.1 ms).
    W = np.einsum("om,omn->on", dw, sw)            # (o, n)
    WT = np.ascontiguousarray(W.T)                 # (n, o) fp32
    c = np.einsum("om,om->o", dw, sb.sum(-1)).astype(np.float32)[:, None]

    nontrivial_ln = not (np.all(ln_w == 1.0) and np.all(ln_b == 0.0))
    key = bool(nontrivial_ln)
    if key not in _NC_CACHE:
        _NC_CACHE[key] = _build_nc(nontrivial_ln)
    nc = _NC_CACHE[key]

    in_maps = []
    for i in range(N_CORES):
        m = {"x": x16[i * BPC:(i + 1) * BPC], "wt": WT, "c": c}
        if nontrivial_ln:
            m["lnw"] = ln_w[None, :]
            m["lnbe"] = (W.sum(-1)[:, None] * ln_b[None, :]).astype(np.float32)
        in_maps.append(m)

    res = run_bass_kernel_spmd(nc, in_maps, core_ids=list(range(N_CORES)),
                               trace=_trace)
    out = np.concatenate([res.results[i]["out"] for i in range(N_CORES)],
                         axis=0).astype(np.float32)
    if _trace:
        return out, res
    return out


# revision 7
# speedup vs baseline: 1.2966x; 1.1419x over previous
"""Trainium2 Bass kernel for nn_DNM_Conv (LayerNorm -> synapse contraction ->
dendritic weighting -> GELU -> residual multiply).

Algebraic reduction of the reference:
    y = LayerNorm(x)                                  (b, n, d)
    t[b,o,d] = sum_n W[o,n] * y[b,n,d] + c[o]
        where W[o,n] = sum_m dw[o,m]*sw[o,m,n],  c[o] = sum_{m,n} dw[o,m]*sb[o,m,n]
    out = x * (gelu_erf(t) + 1)                       (o == n == 196)

Instead of normalizing x, the per-row LN scale is folded into the weights:
    Wr[o,n] = W[o,n] * rstd[n]        (per batch, tiny 196x196 scale)
    t[o,d]  = (Wr @ x)[o,d] - (Wr @ mu)[o] + c[o]
The mu-correction rides as an extra rhs column (x tile has 769 columns, the
last one holding mu), so one widened matmul produces both t and Wr@mu.

Distribution: data-parallel over batch, 8 batches per core on 8 cores.
Datapath is fp16 end to end (x cast on host, fp32 PSUM accumulation,
fp16 output cast back on host); LN statistics are computed in fp32.
"""

import numpy as np

B, N, D, O, M = 64, 196, 768, 196, 2
N_CORES = 8
BPC = B // N_CORES          # batches per core
NPAIR = BPC // 2            # batch pairs (DMA granularity)
NA, NB = 128, 68            # n partition split
OA, OB = 128, 68            # o partition split
DC = 384                    # matmul moving free-dim chunk
LN_EPS = 1e-5

_NC_CACHE = {}


def _build_nc(nontrivial_ln):
    import concourse.bacc as bacc
    import concourse.tile as tile
    import concourse.bass as bass
    from concourse.tile import add_dep_helper
    from concourse import mybir
    from contextlib import ExitStack

    F32 = mybir.dt.float32
    F16 = mybir.dt.float16
    AF = mybir.ActivationFunctionType
    OP = mybir.AluOpType

    nc = bacc.Bacc()
    x_d = nc.declare_dram_parameter("x", [BPC, N, D], F16, isOutput=False)
    wt_d = nc.declare_dram_parameter("wt", [N, O], F16, isOutput=False)
    c_d = nc.declare_dram_parameter("c", [O, 1], F32, isOutput=False)
    if nontrivial_ln:
        lnw_d = nc.declare_dram_parameter("lnw", [1, D], F32, isOutput=False)
        lnbe_d = nc.declare_dram_parameter("lnbe", [O, D], F32, isOutput=False)
    out_d = nc.declare_dram_parameter("out", [BPC, N, D], F16, isOutput=True)

    x_pair = x_d.ap().rearrange("(q j) n d -> q n j d", j=2)    # (4, 196, 2, 768)
    out_pair = out_d.ap().rearrange("(q j) n d -> q n j d", j=2)

    with tile.TileContext(nc) as tc, ExitStack() as ctx:
        const = ctx.enter_context(tc.tile_pool(name="const", bufs=1))
        xpool = ctx.enter_context(tc.tile_pool(name="xpool", bufs=NPAIR))
        stpool = ctx.enter_context(tc.tile_pool(name="stpool", bufs=BPC))
        wrpool = ctx.enter_context(tc.tile_pool(name="wrpool", bufs=3))
        gpool = ctx.enter_context(tc.tile_pool(name="gpool", bufs=3))
        opool = ctx.enter_context(tc.tile_pool(name="opool", bufs=2))
        psum = ctx.enter_context(tc.tile_pool(name="psum", bufs=2, space="PSUM"))

        # ---- constants ----
        wt_a = const.tile([NA, O], F16, tag="wt_a")
        wt_b = const.tile([NB, O], F16, tag="wt_b")
        nc.sync.dma_start(out=wt_a[:], in_=wt_d[0:NA, :])
        nc.sync.dma_start(out=wt_b[:], in_=wt_d[NA:N, :])
        c_a = const.tile([OA, 1], F32, tag="c_a")
        c_b = const.tile([OB, 1], F32, tag="c_b")
        nc.sync.dma_start(out=c_a[:], in_=c_d[0:OA, :])
        nc.sync.dma_start(out=c_b[:], in_=c_d[OA:O, :])
        eps_t = const.tile([128, 1], F32, tag="eps")
        nc.vector.memset(eps_t[:], LN_EPS)
        if nontrivial_ln:
            lnw_t = const.tile([128, D], F32, tag="lnw")
            lnw_bcast = bass.AP(tensor=lnw_d.ap().tensor, offset=0,
                                ap=[[0, 128], [1, D]])
            nc.sync.dma_start(out=lnw_t[:], in_=lnw_bcast)
            lnbe_a = const.tile([OA, D], F32, tag="lnbe_a")
            lnbe_b = const.tile([OB, D], F32, tag="lnbe_b")
            nc.sync.dma_start(out=lnbe_a[:], in_=lnbe_d[0:OA, :])
            nc.sync.dma_start(out=lnbe_b[:], in_=lnbe_d[OA:O, :])

        nsplit = ((0, NA), (NA, NB))

        # ---- phase A: load x (paired, fp16, 769th column reserved for mu*rstd),
        #      bn stats ----
        xtiles = []  # [pair][ci] -> (pn, 2, 769) fp16
        mvs = []     # [batch][ci] -> (pn, 2) fp32 mean/var
        for q in range(NPAIR):
            xq = []
            for ci, (p0, pn) in enumerate(nsplit):
                xt = xpool.tile([pn, 2, D + 2], F16, tag=f"x{ci}")
                nc.sync.dma_start(out=xt[:, :, 0:D],
                                    in_=x_pair[q, p0:p0 + pn, :, :])
                xq.append(xt)
            xtiles.append(xq)
        for i in range(BPC):
            q, j = divmod(i, 2)
            mvi = []
            for ci, (p0, pn) in enumerate(nsplit):
                xt = xtiles[q][ci]
                stats = stpool.tile([pn, 2, 6], F32, tag=f"stats{ci}")
                xg = xt[:, j, 0:D].rearrange("p (s f) -> p s f", s=2)
                for s in range(2):
                    nc.vector.bn_stats(out=stats[:, s, :], in_=xg[:, s, :])
                mv = stpool.tile([pn, 2], F32, tag=f"mv{ci}")
                nc.vector.bn_aggr(out=mv[:], in_=stats[:])
                mvi.append(mv)
            mvs.append(mvi)

        # ---- rstd for all batches (single ACT table-set period) ----
        rstds = []
        rstd_insts = []
        for i in range(BPC):
            ri = []
            for ci, (p0, pn) in enumerate(nsplit):
                rstd = stpool.tile([pn, 1], F32, tag=f"rstd{ci}")
                ins = nc.scalar.activation(out=rstd[:], in_=mvs[i][ci][:, 1:2],
                                           func=AF.Abs_reciprocal_sqrt,
                                           bias=eps_t[0:pn, :], scale=1.0)
                rstd_insts.append(ins)
                ri.append(rstd)
            rstds.append(ri)

        # mu*rstd into the 769th x column (fp16), and Wr = wt * rstd (fp16)
        wrs = []
        for i in range(BPC):
            q, j = divmod(i, 2)
            wri = []
            for ci, (p0, pn) in enumerate(nsplit):
                nc.scalar.activation(out=xtiles[q][ci][:, j, D:D + 1],
                                     in_=mvs[i][ci][:, 0:1],
                                     func=AF.Copy, scale=rstds[i][ci][:])
                wr = wrpool.tile([pn, O], F16, tag=f"wr{ci}")
                wt_t = wt_a if ci == 0 else wt_b
                nc.vector.tensor_scalar_mul(out=wr[:], in0=wt_t[:],
                                            scalar1=rstds[i][ci][:])
                wri.append(wr)
            wrs.append(wri)

        # ---- phase B: matmul + gelu + residual multiply ----
        osplit = ((0, OA, c_a), (OA, OB, c_b))
        first_gelu_logged = False
        outs = None
        for i in range(BPC):
            q, j = divmod(i, 2)
            xs = xtiles[q]
            if j == 0:
                out_a = opool.tile([NA, 2, D], F16, tag="out0")
                out_b = opool.tile([NB, 2, D], F16, tag="out1")
                outs = (out_a, out_b)

            for oc, (o0, on, c_t) in enumerate(osplit):
                # dc=1 widened matmul first: columns 384..768 plus the mu column
                pm1 = psum.tile([on, DC + 1], F32, tag=f"pm{oc}1")
                for k, wr in enumerate(wrs[i]):
                    nc.tensor.matmul(pm1[:], wr[:, o0:o0 + on],
                                     xs[k][:, j, DC:D + 1],
                                     start=(k == 0), stop=(k == 1))
                gbias = stpool.tile([on, 1], F32, tag=f"gb{oc}")
                nc.scalar.activation(out=gbias[:], in_=pm1[:, DC:DC + 1],
                                     func=AF.Identity, bias=c_t[:], scale=-1.0)
                pm0 = psum.tile([on, DC], F32, tag=f"pm{oc}0")
                for k, wr in enumerate(wrs[i]):
                    nc.tensor.matmul(pm0[:], wr[:, o0:o0 + on],
                                     xs[k][:, j, 0:DC],
                                     start=(k == 0), stop=(k == 1))

                for dc, pm in ((1, pm1), (0, pm0)):
                    ds = slice(dc * DC, (dc + 1) * DC)
                    if nontrivial_ln:
                        lnbe_t = lnbe_a if oc == 0 else lnbe_b
                        # t = lnw * (pm - Wr@mu) + lnbe + c
                        nc.vector.tensor_scalar_sub(out=pm[:, 0:DC],
                                                    in0=pm[:, 0:DC],
                                                    scalar1=pm1[:, DC:DC + 1])
                        nc.vector.tensor_mul(out=pm[:, 0:DC], in0=pm[:, 0:DC],
                                             in1=lnw_t[0:on, ds])
                        nc.vector.tensor_add(out=pm[:, 0:DC], in0=pm[:, 0:DC],
                                             in1=lnbe_t[:, ds])
                        g = gpool.tile([on, DC], F16, tag=f"g{oc}{dc}")
                        ins = nc.scalar.activation(out=g[:], in_=pm[:, 0:DC],
                                                   func=AF.Gelu, bias=c_t[:],
                                                   scale=1.0)
                    else:
                        g = gpool.tile([on, DC], F16, tag=f"g{oc}{dc}")
                        ins = nc.scalar.activation(out=g[:], in_=pm[:, 0:DC],
                                                   func=AF.Gelu, bias=gbias[:],
                                                   scale=1.0)
                    if not first_gelu_logged:
                        first_gelu_logged = True
                        add_dep_helper(ins.ins, rstd_insts[-1].ins, sync=True,
                                       reason="group sqrt-set before gelu-set")
                    # out = (g + 1) * x
                    out_t = outs[oc]
                    if oc == 1 and dc == 0:
                        nc.gpsimd.tensor_mul(out=out_t[:, j, ds], in0=g[:],
                                             in1=xs[oc][:, j, ds])
                        nc.gpsimd.tensor_add(out=out_t[:, j, ds],
                                             in0=out_t[:, j, ds],
                                             in1=xs[oc][:, j, ds])
                    else:
                        nc.vector.scalar_tensor_tensor(
                            out=out_t[:, j, ds], in0=g[:], scalar=1.0,
                            in1=xs[oc][:, j, ds], op0=OP.add, op1=OP.mult)

            if j == 1:
                for ci, (p0, pn) in enumerate(nsplit):
                    nc.sync.dma_start(out=out_pair[q, p0:p0 + pn, :, :],
                                      in_=outs[ci][:])

    nc.compile()
    return nc


def kernel(x, ln_w, ln_b, sw, sb, dw, _trace=False):
    from concourse.bass_utils import run_bass_kernel_spmd

    x = np.asarray(x, dtype=np.float32)
    ln_w = np.asarray(ln_w, dtype=np.float32)
    ln_b = np.asarray(ln_b, dtype=np.float32)
    sw = np.asarray(sw, dtype=np.float32)
    sb = np.asarray(sb, dtype=np.float32)
    dw = np.asarray(dw, dtype=np.float32)

    x16 = np.ascontiguousarray(x.astype(np.float16))

    # Fold dendritic weights into the synapse contraction (host, ~0.1 ms).
    W = np.einsum("om,omn->on", dw, sw)            # (o, n)
    WT = np.ascontiguousarray(W.T.astype(np.float16))
    c = np.einsum("om,om->o", dw, sb.sum(-1)).astype(np.float32)[:, None]

    nontrivial_ln = not (np.all(ln_w == 1.0) and np.all(ln_b == 0.0))
    key = bool(nontrivial_ln)
    if key not in _NC_CACHE:
        _NC_CACHE[key] = _build_nc(nontrivial_ln)
    nc = _NC_CACHE[key]

    in_maps = []
    for i in range(N_CORES):
        m = {"x": x16[i * BPC:(i + 1) * BPC], "wt": WT, "c": c}
        if nontrivial_ln:
            m["lnw"] = ln_w[None, :]
            m["lnbe"] = (W.sum(-1)[:, None] * ln_b[None, :]).astype(np.float32)
        in_maps.append(m)

    res = run_bass_kernel_spmd(nc, in_maps, core_ids=list(range(N_CORES)),
                               trace=_trace)
    out = np.concatenate([res.results[i]["out"] for i in range(N_CORES)],
                         axis=0).astype(np.float32)
    if _trace:
        return out, res
    return out


# revision 10
# speedup vs baseline: 1.3326x; 1.0277x over previous
"""Trainium2 Bass kernel for nn_DNM_Conv (LayerNorm -> synapse contraction ->
dendritic weighting -> GELU -> residual multiply).

Algebraic reduction of the reference:
    y = LayerNorm(x)                                  (b, n, d)
    t[b,o,d] = sum_n W[o,n] * y[b,n,d] + c[o]
        where W[o,n] = sum_m dw[o,m]*sw[o,m,n],  c[o] = sum_{m,n} dw[o,m]*sb[o,m,n]
    out = x * (gelu_erf(t) + 1)                       (o == n == 196)

Instead of normalizing x, the per-row LN scale is folded into the weights:
    Wr[o,n] = W[o,n] * rstd[n]        (per batch, tiny 196x196 scale)
    t[o,d]  = (Wr @ x)[o,d] - (Wr @ mu)[o] + c[o]
The mu-correction rides as an extra rhs column (x tile has 769 columns, the
last one holding mu), so one widened matmul produces both t and Wr@mu.

Distribution: data-parallel over batch, 8 batches per core on 8 cores.
Datapath is fp16 end to end (x cast on host, fp32 PSUM accumulation,
fp16 output cast back on host); LN statistics are computed in fp32.
"""

import numpy as np

B, N, D, O, M = 64, 196, 768, 196, 2
N_CORES = 8
BPC = B // N_CORES          # batches per core
NPAIR = BPC // 2            # batch pairs (DMA granularity)
NA, NB = 128, 68            # n partition split
OA, OB = 128, 68            # o partition split
DC = 384                    # matmul moving free-dim chunk
LN_EPS = 1e-5

_NC_CACHE = {}


def _build_nc(nontrivial_ln):
    import concourse.bacc as bacc
    import concourse.tile as tile
    import concourse.bass as bass
    from concourse.tile import add_dep_helper
    from concourse import mybir
    from contextlib import ExitStack

    F32 = mybir.dt.float32
    F16 = mybir.dt.float16
    AF = mybir.ActivationFunctionType
    OP = mybir.AluOpType

    nc = bacc.Bacc()
    x_d = nc.declare_dram_parameter("x", [BPC, N, D], F16, isOutput=False)
    wt_d = nc.declare_dram_parameter("wt", [N, O], F16, isOutput=False)
    c_d = nc.declare_dram_parameter("c", [O, 1], F32, isOutput=False)
    if nontrivial_ln:
        lnw_d = nc.declare_dram_parameter("lnw", [1, D], F32, isOutput=False)
        lnbe_d = nc.declare_dram_parameter("lnbe", [O, D], F32, isOutput=False)
    out_d = nc.declare_dram_parameter("out", [BPC, N, D], F16, isOutput=True)

    x_pair = x_d.ap().rearrange("(q j) n d -> q n j d", j=2)    # (4, 196, 2, 768)
    out_pair = out_d.ap().rearrange("(q j) n d -> q n j d", j=2)

    with tile.TileContext(nc) as tc, ExitStack() as ctx:
        const = ctx.enter_context(tc.tile_pool(name="const", bufs=1))
        xpool = ctx.enter_context(tc.tile_pool(name="xpool", bufs=NPAIR))
        stpool = ctx.enter_context(tc.tile_pool(name="stpool", bufs=BPC))
        wrpool = ctx.enter_context(tc.tile_pool(name="wrpool", bufs=3))
        gpool = ctx.enter_context(tc.tile_pool(name="gpool", bufs=3))
        opool = ctx.enter_context(tc.tile_pool(name="opool", bufs=2))
        psum = ctx.enter_context(tc.tile_pool(name="psum", bufs=2, space="PSUM"))

        # ---- constants ----
        wt_a = const.tile([NA, O], F16, tag="wt_a")
        wt_b = const.tile([NB, O], F16, tag="wt_b")
        nc.sync.dma_start(out=wt_a[:], in_=wt_d[0:NA, :])
        nc.sync.dma_start(out=wt_b[:], in_=wt_d[NA:N, :])
        c_a = const.tile([OA, 1], F32, tag="c_a")
        c_b = const.tile([OB, 1], F32, tag="c_b")
        nc.sync.dma_start(out=c_a[:], in_=c_d[0:OA, :])
        nc.sync.dma_start(out=c_b[:], in_=c_d[OA:O, :])
        eps_t = const.tile([128, 1], F32, tag="eps")
        nc.vector.memset(eps_t[:], LN_EPS)
        if nontrivial_ln:
            lnw_t = const.tile([128, D], F32, tag="lnw")
            lnw_bcast = bass.AP(tensor=lnw_d.ap().tensor, offset=0,
                                ap=[[0, 128], [1, D]])
            nc.sync.dma_start(out=lnw_t[:], in_=lnw_bcast)
            lnbe_a = const.tile([OA, D], F32, tag="lnbe_a")
            lnbe_b = const.tile([OB, D], F32, tag="lnbe_b")
            nc.sync.dma_start(out=lnbe_a[:], in_=lnbe_d[0:OA, :])
            nc.sync.dma_start(out=lnbe_b[:], in_=lnbe_d[OA:O, :])

        nsplit = ((0, NA), (NA, NB))

        # ---- phase A: load x (paired, fp16, 769th column reserved for mu*rstd),
        #      bn stats ----
        xtiles = []  # [pair][ci] -> (pn, 2, 770) fp16
        for q in range(NPAIR):
            xq = []
            for ci, (p0, pn) in enumerate(nsplit):
                xt = xpool.tile([pn, 2, D + 2], F16, tag=f"x{ci}")
                nc.sync.dma_start(out=xt[:, :, 0:D],
                                    in_=x_pair[q, p0:p0 + pn, :, :])
                xq.append(xt)
            xtiles.append(xq)
        for i in range(BPC):
            q, j = divmod(i, 2)
            for ci, (p0, pn) in enumerate(nsplit):
                xt = xtiles[q][ci]
                stats = stpool.tile([pn, 2, 6], F32, tag=f"stats{ci}")
                xg = xt[:, j, 0:D].rearrange("p (s f) -> p s f", s=2)
                for s in range(2):
                    nc.vector.bn_stats(out=stats[:, s, :], in_=xg[:, s, :])
                # write [mean, var] (fp16) straight into x columns 768:770 --
                # the mean becomes the extra matmul-rhs column, var feeds rstd
                nc.vector.bn_aggr(out=xt[:, j, D:D + 2], in_=stats[:])

        # ---- rstd for all batches (single ACT table-set period) ----
        rstds = []
        rstd_insts = []
        for i in range(BPC):
            q, j = divmod(i, 2)
            ri = []
            for ci, (p0, pn) in enumerate(nsplit):
                rstd = stpool.tile([pn, 1], F32, tag=f"rstd{ci}")
                ins = nc.scalar.activation(out=rstd[:],
                                           in_=xtiles[q][ci][:, j, D + 1:D + 2],
                                           func=AF.Abs_reciprocal_sqrt,
                                           bias=eps_t[0:pn, :], scale=1.0)
                rstd_insts.append(ins)
                ri.append(rstd)
            rstds.append(ri)

        # Wr = wt * rstd (fp16); a-tile on ACT, b-tile on DVE
        wrs = []
        for i in range(BPC):
            wri = []
            for ci, (p0, pn) in enumerate(nsplit):
                wr = wrpool.tile([pn, O], F16, tag=f"wr{ci}")
                wt_t = wt_a if ci == 0 else wt_b
                if ci == 0:
                    nc.scalar.activation(out=wr[:], in_=wt_t[:], func=AF.Copy,
                                         scale=rstds[i][ci][:])
                else:
                    nc.vector.tensor_scalar_mul(out=wr[:], in0=wt_t[:],
                                                scalar1=rstds[i][ci][:])
                wri.append(wr)
            wrs.append(wri)

        # ---- phase B: matmul + gelu (per batch), then pair-wide residual
        #      multiply + store ----
        osplit = ((0, OA, c_a), (OA, OB, c_b))
        first_gelu_logged = False
        for q in range(NPAIR):
            xs = xtiles[q]
            out_a = opool.tile([NA, 2, D], F16, tag="out0")
            out_b = opool.tile([NB, 2, D], F16, tag="out1")
            outs = (out_a, out_b)
            # g[oc][dc] spans both batches of the pair: (on, 2, DC)
            gt = [[gpool.tile([on, 2, DC], F16, tag=f"g{oc}{dc}", name=f"g{oc}{dc}")
                   for dc in range(2)] for oc, (o0, on, c_t) in enumerate(osplit)]

            for j in range(2):
                i = 2 * q + j
                for oc, (o0, on, c_t) in enumerate(osplit):
                    # dc=1 widened matmul first: cols 384..768 plus the mu column
                    pm1 = psum.tile([on, DC + 1], F32, tag=f"pm{oc}1")
                    for k, wr in enumerate(wrs[i]):
                        nc.tensor.matmul(pm1[:], wr[:, o0:o0 + on],
                                         xs[k][:, j, DC:D + 1],
                                         start=(k == 0), stop=(k == 1))
                    gbias = stpool.tile([on, 1], F32, tag=f"gb{oc}")
                    nc.vector.tensor_tensor(out=gbias[:], in0=c_t[:],
                                            in1=pm1[:, DC:DC + 1],
                                            op=OP.subtract)
                    pm0 = psum.tile([on, DC], F32, tag=f"pm{oc}0")
                    for k, wr in enumerate(wrs[i]):
                        nc.tensor.matmul(pm0[:], wr[:, o0:o0 + on],
                                         xs[k][:, j, 0:DC],
                                         start=(k == 0), stop=(k == 1))

                    for dc, pm in ((1, pm1), (0, pm0)):
                        ds = slice(dc * DC, (dc + 1) * DC)
                        if nontrivial_ln:
                            lnbe_t = lnbe_a if oc == 0 else lnbe_b
                            # t = lnw * (pm - Wr@mu) + lnbe + c
                            nc.vector.tensor_scalar_sub(
                                out=pm[:, 0:DC], in0=pm[:, 0:DC],
                                scalar1=pm1[:, DC:DC + 1])
                            nc.vector.tensor_mul(out=pm[:, 0:DC],
                                                 in0=pm[:, 0:DC],
                                                 in1=lnw_t[0:on, ds])
                            nc.vector.tensor_add(out=pm[:, 0:DC],
                                                 in0=pm[:, 0:DC],
                                                 in1=lnbe_t[:, ds])
                            ins = nc.scalar.activation(
                                out=gt[oc][dc][:, j, :], in_=pm[:, 0:DC],
                                func=AF.Gelu, bias=c_t[:], scale=1.0)
                        else:
                            ins = nc.scalar.activation(
                                out=gt[oc][dc][:, j, :], in_=pm[:, 0:DC],
                                func=AF.Gelu, bias=gbias[:], scale=1.0)
                        if not first_gelu_logged:
                            first_gelu_logged = True
                            add_dep_helper(ins.ins, rstd_insts[-1].ins,
                                           sync=True,
                                           reason="sqrt-set before gelu-set")

            # pair-wide residual multiply: out = (g + 1) * x
            for oc, (o0, on, c_t) in enumerate(osplit):
                for dc in range(2):
                    ds = slice(dc * DC, (dc + 1) * DC)
                    g2 = gt[oc][dc]
                    if oc == 1 and dc == 0:
                        nc.gpsimd.tensor_mul(out=outs[oc][:, :, ds],
                                             in0=g2[:],
                                             in1=xs[oc][:, :, ds])
                        nc.gpsimd.tensor_add(out=outs[oc][:, :, ds],
                                             in0=outs[oc][:, :, ds],
                                             in1=xs[oc][:, :, ds])
                    else:
                        nc.vector.scalar_tensor_tensor(
                            out=outs[oc][:, :, ds], in0=g2[:], scalar=1.0,
                            in1=xs[oc][:, :, ds], op0=OP.add, op1=OP.mult)

            for ci, (p0, pn) in enumerate(nsplit):
                nc.sync.dma_start(out=out_pair[q, p0:p0 + pn, :, :],
                                  in_=outs[ci][:])

    nc.compile()
    return nc


def kernel(x, ln_w, ln_b, sw, sb, dw, _trace=False):
    from concourse.bass_utils import run_bass_kernel_spmd

    x = np.asarray(x, dtype=np.float32)
    ln_w = np.asarray(ln_w, dtype=np.float32)
    ln_b = np.asarray(ln_b, dtype=np.float32)
    sw = np.asarray(sw, dtype=np.float32)
    sb = np.asarray(sb, dtype=np.float32)
    dw = np.asarray(dw, dtype=np.float32)

    x16 = np.ascontiguousarray(x.astype(np.float16))

    # Fold dendritic weights into the synapse contraction (host, ~0.1 ms).
    W = np.einsum("om,omn->on", dw, sw)            # (o, n)
    WT = np.ascontiguousarray(W.T.astype(np.float16))
    c = np.einsum("om,om->o", dw, sb.sum(-1)).astype(np.float32)[:, None]

    nontrivial_ln = not (np.all(ln_w == 1.0) and np.all(ln_b == 0.0))
    key = bool(nontrivial_ln)
    if key not in _NC_CACHE:
        _NC_CACHE[key] = _build_nc(nontrivial_ln)
    nc = _NC_CACHE[key]

    in_maps = []
    for i in range(N_CORES):
        m = {"x": x16[i * BPC:(i + 1) * BPC], "wt": WT, "c": c}
        if nontrivial_ln:
            m["lnw"] = ln_w[None, :]
            m["lnbe"] = (W.sum(-1)[:, None] * ln_b[None, :]).astype(np.float32)
        in_maps.append(m)

    res = run_bass_kernel_spmd(nc, in_maps, core_ids=list(range(N_CORES)),
                               trace=_trace)
    out = np.concatenate([res.results[i]["out"] for i in range(N_CORES)],
                         axis=0).astype(np.float32)
    if _trace:
        return out, res
    return out
